# revision 6
# baseline (speedup 1.0000x reference)
"""Conv2Demod (StyleGAN modulated conv) via full 2D Winograd F(2x2,3x3) on
Trainium2.

Math restructure vs the direct algorithm:
  conv(weight * style[ci], x) == conv(weight, style[ci] * x)
so style modulation is applied to the input image (per channel) and the conv
weights become sample-independent; the demodulation coefficient is folded into
the PSUM->SBUF evacuation (per-partition ACT scale).

The 3x3 conv runs as 2D Winograd F(2x2,3x3): 16 pointwise products per 2x2
output tile vs 36 for direct = 4/9 the PE MACs. The transformed weights
U2[p][q] = G w G^T are built on the HOST (f64 -> bf16) since they are
sample-independent; styles and dcoefs are also host-computed (tiny GEMMs).

Per (sample, 16-row chunk), all tensors [128 part, free]:
  band   : DMA 18 rows x [E,O,E+1,O+1] parity planes (4B-aligned DVE reads)
  scale  : band *= styles[ci]                        (DVE tensor_scalar, 4x)
  in-h   : 4 horizontal B^T combos -> hq[q]          (DVE 2x)
  in-v   : 4 vertical B^T combos -> v[p] (all q)     (DVE 2x)
  matmul : ps4[p] = sum_ci U2[p,q][ci,:].T @ v[p,ci,q]   (PE, FD=256,
           16 groups of 16 MMs; quad of p shares a 2-bank PSUM tile)
  evac   : ps4 -> m_sb bf16 * dcoef[co]              (ACT, 1024 el/op)
  out-s1 : r0/r1 = A^T over p                        (DVE 2x)
  out-s2 : y    = A^T over q                         (GpSimd)
Host does layout only otherwise: parity split of the image, U2 transform,
and the final untiling of the output.
"""

import numpy as np
import ml_dtypes

import concourse.bass as bass
import concourse.tile as tile
from concourse import bacc, mybir
from concourse.bass import ts
from concourse.bass_utils import run_bass_kernel_spmd

N_CORES = 8
B_SZ, C, Z, K, H, W = 16, 512, 512, 3, 64, 64
S = B_SZ // N_CORES            # samples per core
P = 128
NT = C // P                    # channel tiles
EPS = 1e-8

ROWS = H + 2                   # padded rows
PW = 34                        # parity-split padded width
TC = W // 2                    # winograd tile columns (32)
CH = 4                         # 16-row chunks per sample
RC = H // CH                   # output rows per chunk (16)
RB = RC + 2                    # band rows per chunk (18)
R2 = RB // 2                   # band row pairs (9)
TR = RC // 2                   # winograd tile rows per chunk (8)
FD2 = TR * TC                  # matmul free dim (256)

BF16 = mybir.dt.bfloat16
F32 = mybir.dt.float32

LAST_RESULT = None
_NC_CACHE = {}


def _build_nc():
    nc = bacc.Bacc(None)

    xp2 = nc.dram_tensor("xp2", [S, C, ROWS, 4, PW], BF16, kind="ExternalInput")
    wU2 = nc.dram_tensor("wU2", [4, 4, C, C], BF16, kind="ExternalInput")
    styT = nc.dram_tensor("styT", [C, S], F32, kind="ExternalInput")
    dcoT = nc.dram_tensor("dcoT", [C, S], F32, kind="ExternalInput")
    out = nc.dram_tensor("out", [S, CH, 2, 2, P, NT, FD2], BF16,
                         kind="ExternalOutput")

    xp2_r = xp2.rearrange("s (t p) r q c -> s t p (r q c)", p=P)
    wU2_r = wU2.rearrange("a b (t p) c -> a b t p c", p=P)
    styT_r = styT.rearrange("(t p) s -> t p s", p=P)
    dcoT_r = dcoT.rearrange("(t p) s -> t p s", p=P)

    with tile.TileContext(nc) as tc:
        with (
            tc.tile_pool(name="persist", bufs=1) as persist,
            tc.tile_pool(name="bandp", bufs=1) as bandp,
            tc.tile_pool(name="hqp", bufs=2) as hqp,
            tc.tile_pool(name="vp", bufs=2) as vp,
            tc.tile_pool(name="mp", bufs=2) as mp,
            tc.tile_pool(name="rp", bufs=1) as rp,
            tc.tile_pool(name="yp", bufs=2) as yp,
            tc.tile_pool(name="psum", bufs=4, space="PSUM") as psum,
        ):
            # ---------- params ----------
            sty = [persist.tile([P, S], F32, tag=f"sty{t}", name=f"sty{t}")
                   for t in range(NT)]
            dco = [persist.tile([P, S], F32, tag=f"dco{t}", name=f"dco{t}")
                   for t in range(NT)]
            for t in range(NT):
                nc.sync.dma_start(out=sty[t], in_=styT_r[t])
                nc.sync.dma_start(out=dco[t], in_=dcoT_r[t])

            # ---------- U2 weights (q-major DMA order so q=0 lands first) ----
            u2 = {}
            for q in range(4):
                for p_ in range(4):
                    for ci in range(NT):
                        wt = persist.tile([P, C], BF16, tag=f"u2_{p_}_{q}_{ci}",
                                          name=f"u2_{p_}_{q}_{ci}")
                        nc.sync.dma_start(out=wt, in_=wU2_r[p_][q][ci])
                        u2[(p_, q, ci)] = wt

            # ---------- per-chunk stages ----------
            V = {}   # (p, ci) -> current v tile

            def emit_band_dma(s, k):
                bts = []
                for ci in range(NT):
                    bt = bandp.tile([P, R2, 2, 4, PW], BF16,
                                    tag=f"band{ci}", name=f"band{ci}")
                    row0 = RC * k
                    nc.gpsimd.dma_start(
                        out=bt,
                        in_=xp2_r[s][ci][:, row0 * 4 * PW:
                                         (row0 + RB) * 4 * PW])
                    bts.append(bt)
                return bts

            def emit_input_ci(s, k, bts, ci):
                """scale (ACT) + in-h + in-v for one ci of chunk (s,k)."""
                bt = bts[ci]
                btf = bt.rearrange("p a b c d -> p (a b c d)")
                nc.scalar.activation(
                    btf, btf, mybir.ActivationFunctionType.Copy,
                    scale=sty[ci][:, s:s + 1])
                hq = hqp.tile([P, 4, R2, 2, TC], BF16, tag="hq",
                              name=f"hq{ci}")
                xE = bt[:, :, :, 0, 0:TC]
                xO = bt[:, :, :, 1, 0:TC]
                xE1 = bt[:, :, :, 2, 0:TC]
                xO1 = bt[:, :, :, 3, 0:TC]
                nc.vector.tensor_sub(hq[:, 0], xE, xE1)    # q0
                nc.vector.tensor_add(hq[:, 1], xO, xE1)    # q1
                nc.vector.tensor_sub(hq[:, 2], xE1, xO)    # q2
                nc.vector.tensor_sub(hq[:, 3], xO, xO1)    # q3
                # in-v: v[p][q,tr,tc], all 4 q per op
                h_a = hq[:, :, 0:TR, 0, :]       # row 2tr
                h_b = hq[:, :, 0:TR, 1, :]       # row 2tr+1
                h_c = hq[:, :, 1:TR + 1, 0, :]   # row 2tr+2
                h_d = hq[:, :, 1:TR + 1, 1, :]   # row 2tr+3
                for p_ in range(4):
                    v = vp.tile([P, 4, TR, TC], BF16, tag=f"v{p_}_{ci}",
                                name=f"v{p_}_{ci}")
                    if p_ == 0:
                        nc.vector.tensor_sub(v[:], h_a, h_c)
                    elif p_ == 1:
                        nc.vector.tensor_add(v[:], h_b, h_c)
                    elif p_ == 2:
                        nc.vector.tensor_sub(v[:], h_c, h_b)
                    else:
                        nc.vector.tensor_sub(v[:], h_b, h_d)
                    V[(p_, ci)] = v

            def emit_qs(s, k, qs, Vcur):
                """MM groups + evac + out-s1 for one q slot."""
                m_sb = mp.tile([P, NT, 4, FD2], BF16, tag="m", name=f"m{qs}")
                for cot in range(NT):
                    ps4 = psum.tile([P, 4, FD2], F32, tag="ps4", name="ps4")
                    for p_ in range(4):
                        for ci in range(NT):
                            nc.tensor.matmul(
                                ps4[:, p_],
                                lhsT=u2[(p_, qs, ci)][:, ts(cot, P)],
                                rhs=Vcur[(p_, ci)][:, qs],
                                start=(ci == 0),
                                stop=(ci == NT - 1),
                                skip_group_check=True,
                            )
                    nc.scalar.activation(
                        m_sb[:, cot], ps4[:, :],
                        mybir.ActivationFunctionType.Copy,
                        scale=dco[cot][:, s:s + 1])
                return m_sb

            def emit_outs1(qs, m_sb, r_cur):
                m0 = m_sb[:, :, 0, :]
                m1 = m_sb[:, :, 1, :]
                m2 = m_sb[:, :, 2, :]
                m3 = m_sb[:, :, 3, :]
                # r[qs] holds both A^T-over-p outputs: [:,0]=r0, [:,1]=r1
                r = rp.tile([P, 2, NT, FD2], BF16, tag=f"r{qs}", name=f"r{qs}")
                nc.vector.tensor_add(r[:, 0], m0, m1)
                nc.vector.tensor_add(r[:, 0], r[:, 0], m2)
                nc.vector.tensor_sub(r[:, 1], m1, m2)
                nc.vector.tensor_sub(r[:, 1], r[:, 1], m3)
                r_cur[qs] = r

            def emit_outs2(s, k, r_cur):
                # A^T over q for both u at once (2048-el GpSimd ops):
                #   ye = r[0]+r[1]+r[2] ; yo = r[1]-r[2]-r[3]
                ye = yp.tile([P, 2, NT, FD2], BF16, tag="ye", name="ye")
                yo = yp.tile([P, 2, NT, FD2], BF16, tag="yo", name="yo")
                nc.gpsimd.tensor_add(ye[:], r_cur[0][:], r_cur[1][:])
                nc.gpsimd.tensor_add(ye[:], ye[:], r_cur[2][:])
                nc.gpsimd.tensor_sub(yo[:], r_cur[1][:], r_cur[2][:])
                nc.gpsimd.tensor_sub(yo[:], yo[:], r_cur[3][:])
                # out[s,k,par] = [2u, P, NT, FD2]
                nc.sync.dma_start(
                    out=out[s, k, 0].rearrange("u p t f -> p u t f"), in_=ye)
                nc.sync.dma_start(
                    out=out[s, k, 1].rearrange("u p t f -> p u t f"), in_=yo)

            # ---------- emission schedule ----------
            chunks = [(s, k) for s in range(S) for k in range(CH)]
            NG = len(chunks)

            band_next = emit_band_dma(*chunks[0])
            for ci in range(NT):
                emit_input_ci(*chunks[0], band_next, ci)
            Vprev = dict(V)
            band_next = emit_band_dma(*chunks[1])

            for g, (s, k) in enumerate(chunks):
                Vcur = Vprev
                r_cur = {}
                for qs in range(4):
                    m_sb = emit_qs(s, k, qs, Vcur)
                    # spread next chunk's input work (one ci per qs slot)
                    if g + 1 < NG:
                        emit_input_ci(*chunks[g + 1], band_next, qs)
                    if qs == 1 and g + 2 < NG:
                        band_next2 = emit_band_dma(*chunks[g + 2])
                    emit_outs1(qs, m_sb, r_cur)
                if g + 1 < NG:
                    Vprev = dict(V)
                if g + 2 < NG:
                    band_next = band_next2
                emit_outs2(s, k, r_cur)

    nc.finalize()
    return nc


def _host_prep(img, weight):
    bf = ml_dtypes.bfloat16
    # shifted parity planes of the SAME-padded image:
    #   plane0 E:  x = 2c   plane1 O:  x = 2c+1
    #   plane2 E1: x = 2c+2 plane3 O1: x = 2c+3   (padded coords)
    xp2 = np.zeros((B_SZ, C, ROWS, 4, PW), dtype=bf)
    imgb = img.astype(bf)
    xp2[:, :, 1:H + 1, 0, 1:33] = imgb[:, :, :, 1::2]
    xp2[:, :, 1:H + 1, 1, 0:32] = imgb[:, :, :, 0::2]
    xp2[:, :, :, 2, 0:PW - 1] = xp2[:, :, :, 0, 1:PW]
    xp2[:, :, :, 3, 0:PW - 1] = xp2[:, :, :, 1, 1:PW]
    # U2[p,q,ci,co] = sum_ab G[p,a] G[q,b] w[co,ci,a,b]  (lhsT layout)
    G = np.array([[1, 0, 0], [.5, .5, .5], [.5, -.5, .5], [0, 0, 1]])
    wU2 = np.einsum('pa,oiab,qb->pqio', G, weight.astype(np.float64), G)
    return xp2, np.ascontiguousarray(wU2.astype(bf))


def _decode_out(raw):
    # raw: [S, CH, 2par, 2u, P, NT, FD2] bf16 -> [S, C, H, W] f32
    y = np.asarray(raw).reshape(S, CH, 2, 2, P, NT, TR, TC).astype(np.float32)
    # res[s, t*128+p, 16k+2tr+u, 2tc+par] = y[s,k,par,u,p,t,tr,tc]
    y = y.transpose(0, 5, 4, 1, 6, 3, 7, 2)   # s t p k tr u tc par
    return y.reshape(S, C, H, W)


def kernel(img, ws, noise, weight, A_w, A_b, B_param):
    global LAST_RESULT
    img = np.asarray(img, dtype=np.float32)
    ws = np.asarray(ws, dtype=np.float32)
    noise = np.asarray(noise, dtype=np.float32)
    weight = np.asarray(weight, dtype=np.float32)
    A_w = np.asarray(A_w, dtype=np.float32)
    A_b = np.asarray(A_b, dtype=np.float32)
    B_param = np.asarray(B_param, dtype=np.float32)

    if "wino2d" not in _NC_CACHE:
        _NC_CACHE["wino2d"] = _build_nc()
    nc = _NC_CACHE["wino2d"]

    xp2, wU2 = _host_prep(img, weight)
    # styles and demod coefficients on host (tiny GEMMs, f64)
    styles = (ws.astype(np.float64) @ A_w.T.astype(np.float64)
              + A_b.astype(np.float64))                       # [B, C_in]
    w2 = (weight.astype(np.float64) ** 2).sum(axis=(2, 3))    # [co, ci]
    dcoefs = 1.0 / np.sqrt(styles ** 2 @ w2.T + EPS)          # [B, co]

    in_maps = []
    for c in range(N_CORES):
        sl = slice(c * S, (c + 1) * S)
        in_maps.append({
            "xp2": np.ascontiguousarray(xp2[sl]),
            "wU2": wU2,
            "styT": np.ascontiguousarray(styles[sl].T.astype(np.float32)),
            "dcoT": np.ascontiguousarray(dcoefs[sl].T.astype(np.float32)),
        })

    res = run_bass_kernel_spmd(nc, in_maps, core_ids=list(range(N_CORES)))
    LAST_RESULT = res
    parts = [_decode_out(res.results[c]["out"]) for c in range(N_CORES)]
    out = np.concatenate(parts, axis=0)

    if np.any(B_param):
        out = out + B_param[None, :, None, None] * noise
    return out


# revision 13
# speedup vs baseline: 1.0098x; 1.0098x over previous
"""Conv2Demod (StyleGAN modulated conv) via full 2D Winograd F(2x2,3x3) on
Trainium2.

Math restructure vs the direct algorithm:
  conv(weight * style[ci], x) == conv(weight, style[ci] * x)
so style modulation is applied to the input image (per channel) and the conv
weights become sample-independent; the demodulation coefficient is folded into
the PSUM->SBUF evacuation (per-partition ACT scale).

The 3x3 conv runs as 2D Winograd F(2x2,3x3): 16 pointwise products per 2x2
output tile vs 36 for direct = 4/9 the PE MACs. The transformed weights
U2[p][q] = G w G^T are built on the HOST (f64 -> bf16) since they are
sample-independent; styles and dcoefs are also host-computed (tiny GEMMs).

Per (sample, 16-row chunk), all tensors [128 part, free]:
  band   : DMA 18 rows x [E,O,E+1,O+1] parity planes (4B-aligned DVE reads)
  scale  : band *= styles[ci]                        (DVE tensor_scalar, 4x)
  in-h   : 4 horizontal B^T combos -> hq[q]          (DVE 2x)
  in-v   : 4 vertical B^T combos -> v[p] (all q)     (DVE 2x)
  matmul : ps4[p] = sum_ci U2[p,q][ci,:].T @ v[p,ci,q]   (PE, FD=256,
           16 groups of 16 MMs; quad of p shares a 2-bank PSUM tile)
  evac   : ps4 -> m_sb bf16 * dcoef[co]              (ACT, 1024 el/op)
  out-s1 : r0/r1 = A^T over p                        (DVE 2x)
  out-s2 : y    = A^T over q                         (GpSimd)
Host does layout only otherwise: parity split of the image, U2 transform,
and the final untiling of the output.
"""

import numpy as np
import ml_dtypes

import concourse.bass as bass
import concourse.tile as tile
from concourse import bacc, mybir
from concourse.bass import ts
from concourse.bass_utils import run_bass_kernel_spmd

N_CORES = 8
B_SZ, C, Z, K, H, W = 16, 512, 512, 3, 64, 64
S = B_SZ // N_CORES            # samples per core
P = 128
NT = C // P                    # channel tiles
EPS = 1e-8

ROWS = H + 2                   # padded rows
PW = 34                        # parity-split padded width
TC = W // 2                    # winograd tile columns (32)
CH = 4                         # 16-row chunks per sample
RC = H // CH                   # output rows per chunk (16)
RB = RC + 2                    # band rows per chunk (18)
R2 = RB // 2                   # band row pairs (9)
TR = RC // 2                   # winograd tile rows per chunk (8)
FD2 = TR * TC                  # matmul free dim (256)

BF16 = mybir.dt.bfloat16
F32 = mybir.dt.float32

LAST_RESULT = None
_NC_CACHE = {}


def _build_nc():
    nc = bacc.Bacc(None)

    xp2 = nc.dram_tensor("xp2", [S, C, ROWS, 4, PW], BF16, kind="ExternalInput")
    wU2 = nc.dram_tensor("wU2", [4, 4, C, C], BF16, kind="ExternalInput")
    styT = nc.dram_tensor("styT", [C, S], F32, kind="ExternalInput")
    dcoT = nc.dram_tensor("dcoT", [C, S], F32, kind="ExternalInput")
    out = nc.dram_tensor("out", [S, CH, 2, 2, P, NT, FD2], BF16,
                         kind="ExternalOutput")

    xp2_r = xp2.rearrange("s (t p) r q c -> s t p (r q c)", p=P)
    wU2_r = wU2.rearrange("a b (t p) c -> a b t p c", p=P)
    styT_r = styT.rearrange("(t p) s -> t p s", p=P)
    dcoT_r = dcoT.rearrange("(t p) s -> t p s", p=P)

    with tile.TileContext(nc) as tc:
        with (
            tc.tile_pool(name="persist", bufs=1) as persist,
            tc.tile_pool(name="bandp", bufs=1) as bandp,
            tc.tile_pool(name="hqp", bufs=1) as hqp,
            tc.tile_pool(name="vp", bufs=2) as vp,
            tc.tile_pool(name="mp", bufs=2) as mp,
            tc.tile_pool(name="rp", bufs=2) as rp,
            tc.tile_pool(name="yp", bufs=1) as yp,
            tc.tile_pool(name="psum", bufs=4, space="PSUM") as psum,
        ):
            # ---------- params ----------
            sty = [persist.tile([P, S], F32, tag=f"sty{t}", name=f"sty{t}")
                   for t in range(NT)]
            dco = [persist.tile([P, S], F32, tag=f"dco{t}", name=f"dco{t}")
                   for t in range(NT)]
            for t in range(NT):
                nc.sync.dma_start(out=sty[t], in_=styT_r[t])
                nc.sync.dma_start(out=dco[t], in_=dcoT_r[t])

            # ---------- U2 weights (q-major DMA order so q=0 lands first) ----
            u2 = {}
            for q in range(4):
                for p_ in range(4):
                    for ci in range(NT):
                        wt = persist.tile([P, C], BF16, tag=f"u2_{p_}_{q}_{ci}",
                                          name=f"u2_{p_}_{q}_{ci}")
                        nc.sync.dma_start(out=wt, in_=wU2_r[p_][q][ci])
                        u2[(p_, q, ci)] = wt

            # ---------- per-chunk stages ----------
            V = {}   # (p, ci) -> current v tile

            def emit_band_dma(s, k):
                bts = []
                for ci in range(NT):
                    bt = bandp.tile([P, R2, 2, 4, PW], BF16,
                                    tag="band", bufs=3, name=f"band{ci}")
                    row0 = RC * k
                    nc.gpsimd.dma_start(
                        out=bt,
                        in_=xp2_r[s][ci][:, row0 * 4 * PW:
                                         (row0 + RB) * 4 * PW])
                    bts.append(bt)
                return bts

            def emit_input_ci(s, k, bts, ci):
                """scale (DVE oop 4x) + in-h + in-v for one ci of chunk."""
                bt = bts[ci]
                sb = bandp.tile([P, R2, 2, 4, TC], BF16, tag="scband",
                                name="scband")
                nc.vector.tensor_scalar_mul(
                    sb[:], bt[:, :, :, :, 0:TC], sty[ci][:, s:s + 1])
                hq = hqp.tile([P, 4, R2, 2, TC], BF16, tag="hq",
                              name=f"hq{ci}")
                xE = sb[:, :, :, 0, :]
                xO = sb[:, :, :, 1, :]
                xE1 = sb[:, :, :, 2, :]
                xO1 = sb[:, :, :, 3, :]
                nc.vector.tensor_sub(hq[:, 0], xE, xE1)    # q0
                nc.vector.tensor_add(hq[:, 1], xO, xE1)    # q1
                nc.vector.tensor_sub(hq[:, 2], xE1, xO)    # q2
                nc.vector.tensor_sub(hq[:, 3], xO, xO1)    # q3
                # in-v: v[p][q,tr,tc], all 4 q per op
                h_a = hq[:, :, 0:TR, 0, :]       # row 2tr
                h_b = hq[:, :, 0:TR, 1, :]       # row 2tr+1
                h_c = hq[:, :, 1:TR + 1, 0, :]   # row 2tr+2
                h_d = hq[:, :, 1:TR + 1, 1, :]   # row 2tr+3
                for p_ in range(4):
                    v = vp.tile([P, 4, TR, TC], BF16, tag=f"v{p_}_{ci}",
                                name=f"v{p_}_{ci}")
                    if p_ == 0:
                        nc.vector.tensor_sub(v[:], h_a, h_c)
                    elif p_ == 1:
                        nc.vector.tensor_add(v[:], h_b, h_c)
                    elif p_ == 2:
                        nc.vector.tensor_sub(v[:], h_c, h_b)
                    else:
                        nc.vector.tensor_sub(v[:], h_b, h_d)
                    V[(p_, ci)] = v

            def emit_qs(s, k, qs, Vcur):
                """MM groups + evac + out-s1 for one q slot."""
                m_sb = mp.tile([P, NT, 4, FD2], BF16, tag="m", name=f"m{qs}")
                for cot in range(NT):
                    ps4 = psum.tile([P, 4, FD2], F32, tag="ps4", name="ps4")
                    for p_ in range(4):
                        for ci in range(NT):
                            nc.tensor.matmul(
                                ps4[:, p_],
                                lhsT=u2[(p_, qs, ci)][:, ts(cot, P)],
                                rhs=Vcur[(p_, ci)][:, qs],
                                start=(ci == 0),
                                stop=(ci == NT - 1),
                                skip_group_check=True,
                            )
                    nc.scalar.activation(
                        m_sb[:, cot], ps4[:, :],
                        mybir.ActivationFunctionType.Copy,
                        scale=dco[cot][:, s:s + 1])
                return m_sb

            def emit_outs1(qs, m_sb, r_cur):
                m0 = m_sb[:, :, 0, :]
                m1 = m_sb[:, :, 1, :]
                m2 = m_sb[:, :, 2, :]
                m3 = m_sb[:, :, 3, :]
                # r[qs] holds both A^T-over-p outputs: [:,0]=r0, [:,1]=r1
                r = rp.tile([P, 2, NT, FD2], BF16, tag=f"r{qs}", name=f"r{qs}")
                nc.vector.tensor_add(r[:, 0], m0, m1)
                nc.vector.tensor_add(r[:, 0], r[:, 0], m2)
                nc.vector.tensor_sub(r[:, 1], m1, m2)
                nc.vector.tensor_sub(r[:, 1], r[:, 1], m3)
                r_cur[qs] = r

            def emit_outs2(s, k, r_cur):
                # A^T over q for both u at once (2048-el GpSimd ops):
                #   ye = r[0]+r[1]+r[2] ; yo = r[1]-r[2]-r[3]
                ye = yp.tile([P, 2, NT, FD2], BF16, tag="ye", name="ye")
                yo = yp.tile([P, 2, NT, FD2], BF16, tag="yo", name="yo")
                nc.gpsimd.tensor_add(ye[:], r_cur[0][:], r_cur[1][:])
                nc.gpsimd.tensor_add(ye[:], ye[:], r_cur[2][:])
                nc.gpsimd.tensor_sub(yo[:], r_cur[1][:], r_cur[2][:])
                nc.gpsimd.tensor_sub(yo[:], yo[:], r_cur[3][:])
                # out[s,k,par] = [2u, P, NT, FD2]
                nc.sync.dma_start(
                    out=out[s, k, 0].rearrange("u p t f -> p u t f"), in_=ye)
                nc.sync.dma_start(
                    out=out[s, k, 1].rearrange("u p t f -> p u t f"), in_=yo)

            # ---------- emission schedule ----------
            chunks = [(s, k) for s in range(S) for k in range(CH)]
            NG = len(chunks)

            band_next = emit_band_dma(*chunks[0])
            for ci in range(NT):
                emit_input_ci(*chunks[0], band_next, ci)
            Vprev = dict(V)
            band_next = emit_band_dma(*chunks[1])

            for g, (s, k) in enumerate(chunks):
                Vcur = Vprev
                r_cur = {}
                for qs in range(4):
                    m_sb = emit_qs(s, k, qs, Vcur)
                    # spread next chunk's input work (one ci per qs slot)
                    if g + 1 < NG:
                        emit_input_ci(*chunks[g + 1], band_next, qs)
                    if qs == 1 and g + 2 < NG:
                        band_next2 = emit_band_dma(*chunks[g + 2])
                    emit_outs1(qs, m_sb, r_cur)
                if g + 1 < NG:
                    Vprev = dict(V)
                if g + 2 < NG:
                    band_next = band_next2
                emit_outs2(s, k, r_cur)

    nc.finalize()
    return nc


def _host_prep(img, weight):
    bf = ml_dtypes.bfloat16
    # shifted parity planes of the SAME-padded image:
    #   plane0 E:  x = 2c   plane1 O:  x = 2c+1
    #   plane2 E1: x = 2c+2 plane3 O1: x = 2c+3   (padded coords)
    xp2 = np.zeros((B_SZ, C, ROWS, 4, PW), dtype=bf)
    imgb = img.astype(bf)
    xp2[:, :, 1:H + 1, 0, 1:33] = imgb[:, :, :, 1::2]
    xp2[:, :, 1:H + 1, 1, 0:32] = imgb[:, :, :, 0::2]
    xp2[:, :, :, 2, 0:PW - 1] = xp2[:, :, :, 0, 1:PW]
    xp2[:, :, :, 3, 0:PW - 1] = xp2[:, :, :, 1, 1:PW]
    # U2[p,q,ci,co] = sum_ab G[p,a] G[q,b] w[co,ci,a,b]  (lhsT layout)
    G = np.array([[1, 0, 0], [.5, .5, .5], [.5, -.5, .5], [0, 0, 1]])
    wU2 = np.einsum('pa,oiab,qb->pqio', G, weight.astype(np.float64), G)
    return xp2, np.ascontiguousarray(wU2.astype(bf))


def _decode_out(raw):
    # raw: [S, CH, 2par, 2u, P, NT, FD2] bf16 -> [S, C, H, W] f32
    y = np.asarray(raw).reshape(S, CH, 2, 2, P, NT, TR, TC).astype(np.float32)
    # res[s, t*128+p, 16k+2tr+u, 2tc+par] = y[s,k,par,u,p,t,tr,tc]
    y = y.transpose(0, 5, 4, 1, 6, 3, 7, 2)   # s t p k tr u tc par
    return y.reshape(S, C, H, W)


def kernel(img, ws, noise, weight, A_w, A_b, B_param):
    global LAST_RESULT
    img = np.asarray(img, dtype=np.float32)
    ws = np.asarray(ws, dtype=np.float32)
    noise = np.asarray(noise, dtype=np.float32)
    weight = np.asarray(weight, dtype=np.float32)
    A_w = np.asarray(A_w, dtype=np.float32)
    A_b = np.asarray(A_b, dtype=np.float32)
    B_param = np.asarray(B_param, dtype=np.float32)

    if "wino2d" not in _NC_CACHE:
        _NC_CACHE["wino2d"] = _build_nc()
    nc = _NC_CACHE["wino2d"]

    xp2, wU2 = _host_prep(img, weight)
    # styles and demod coefficients on host (tiny GEMMs, f64)
    styles = (ws.astype(np.float64) @ A_w.T.astype(np.float64)
              + A_b.astype(np.float64))                       # [B, C_in]
    w2 = (weight.astype(np.float64) ** 2).sum(axis=(2, 3))    # [co, ci]
    dcoefs = 1.0 / np.sqrt(styles ** 2 @ w2.T + EPS)          # [B, co]

    in_maps = []
    for c in range(N_CORES):
        sl = slice(c * S, (c + 1) * S)
        in_maps.append({
            "xp2": np.ascontiguousarray(xp2[sl]),
            "wU2": wU2,
            "styT": np.ascontiguousarray(styles[sl].T.astype(np.float32)),
            "dcoT": np.ascontiguousarray(dcoefs[sl].T.astype(np.float32)),
        })

    res = run_bass_kernel_spmd(nc, in_maps, core_ids=list(range(N_CORES)))
    LAST_RESULT = res
    parts = [_decode_out(res.results[c]["out"]) for c in range(N_CORES)]
    out = np.concatenate(parts, axis=0)

    if np.any(B_param):
        out = out + B_param[None, :, None, None] * noise
    return out
